# revision 20
# baseline (speedup 1.0000x reference)
"""CTPN loss kernel for Trainium2 (8 NeuronCores, Bass/Tile).

Strategy
--------
The loss only touches 64 pos + 64 neg anchor locations of the dense
[1,2K,H,W] maps, so the kernel is a sparse gather + tiny reduction.

- Dense tensors are sharded by image rows into 2 shards of 256 rows;
  each shard is held by 4 cores (data-parallel replicas). scores,
  vcoords and sides shards are concatenated into ONE flat per-core
  "data" tensor so every gathered element addresses a single tensor.
- Anchors are routed to the 4 cores holding their rows (round-robin),
  so each core owns at most 16 pos + 16 neg anchors. All 112 values a
  core needs (5 per pos anchor slot, 2 per neg slot) are fetched from
  HBM with a single [128,1] indirect DMA (partition-major, the only
  layout the hardware DGE gather supports), then pivoted on-chip with
  one DVE 32x32 block transpose: slot block i (32 slots) lands on
  partition 32*i as a free-dim row of 32. Minuends and subtrahends
  share a block, so every downstream op is a plain free-dim slice at a
  legal partition start (0/32/64/96) with uniform offsets.
- Loss terms: CE via softplus(d) = ln(1+exp(d)), smooth-L1 via
  0.5*min(|d|,1)^2 + |d| - min(|d|,1). Terms are weighted (anchor
  validity / side mask) and block-reduced with two 16-wide X-reduces
  into 8 partial sums per core. The 8 partial vectors are summed on
  host (the all-reduce of the scalar losses) and normalized into the
  4 outputs.

Gather slot layout (16 slots per half-block; block i -> partition 32i):
  [0:16)    s0p   [16:32)  s1p     (pos CE pair        -> row 0)
  [32:48)   s1n   [48:64)  s0n     (neg CE pair        -> row 32)
  [64:80)   v0    [80:96)  v1      (v-coord minuends   -> row 64)
  [96:112)  sd    [112:128) pad    (side minuends/cnt  -> row 96)

The compiled program is input-independent (offsets travel as data), so
it is built and compiled once and reused across calls.
"""

import os

import numpy as np

H, W, K = 512, 1024, 10
N_POS = 64
N_NEG = 64
N_CORES = 8
N_SHARDS = 2
LANES = N_CORES // N_SHARDS  # replicas per shard
HSH = H // N_SHARDS          # 256 rows per shard
PLANE = HSH * W              # elements per channel per shard
SCB = 0                      # scores base within concat data
VCB = 2 * K * PLANE          # vcoords base
SDB = 4 * K * PLANE          # sides base
DN = 5 * K * PLANE           # total concat length per core
CAP = 16                     # per-core anchor capacity (exact worst case)

_CACHE = {}
last_exec_time_ns = None


def _build_nc():
    import concourse.bass as bass
    import concourse.tile as tile
    from concourse import bacc, mybir
    from contextlib import ExitStack

    f32 = mybir.dt.float32
    i32 = mybir.dt.int32
    AF = mybir.ActivationFunctionType
    ALU = mybir.AluOpType

    nc = bacc.Bacc(
        "TRN2", target_bir_lowering=False, debug=False, num_devices=N_CORES
    )

    data = nc.dram_tensor("data", [DN, 1], f32, kind="ExternalInput")
    idx = nc.dram_tensor("idx", [128, 1], i32, kind="ExternalInput")
    # hostdat rows = partitions; data rows {0,32,64,96}, cols [0:32) targets,
    # [32:64) weights (aligned with the transposed data rows), zero elsewhere
    hd = nc.dram_tensor("hostdat", [128, 64], f32, kind="ExternalInput")
    out = nc.dram_tensor("partials", [4, 2], f32, kind="ExternalOutput")

    with ExitStack() as ctx:
        tc = ctx.enter_context(tile.TileContext(nc))
        pool = ctx.enter_context(tc.tile_pool(name="p", bufs=1))

        idx_t = pool.tile([128, 1], i32)
        # gpsimd-issued load: gpsimd is the gather engine, so the index tile
        # is ready on it as early as possible
        nc.gpsimd.dma_start(idx_t[:], idx.ap())
        HDt = pool.tile([128, 64], f32)
        nc.sync.dma_start(HDt[:], hd.ap())

        A32 = pool.tile([128, 32], f32)
        nc.vector.memset(A32[:, 1:32], 0.0)
        nc.gpsimd.indirect_dma_start(
            out=A32[:, 0:1],
            out_offset=None,
            in_=data.ap(),
            in_offset=bass.IndirectOffsetOnAxis(ap=idx_t[:], axis=0),
        )
        # pivot: 32x32 block transpose puts slot block i on partition 32i
        # (all other rows of At are exact zeros, so batched ops over the
        # full [0:64) / [64:128) partition ranges stay finite and the
        # zero weights kill their contributions)
        At = pool.tile([128, 32], f32)
        nc.vector.transpose(At[:], A32[:])

        Dt = pool.tile([128, 32], f32)
        # CE diffs: minuend - subtrahend halves of rows 0 and 32
        nc.vector.tensor_tensor(
            out=Dt[0:64, 0:16],
            in0=At[0:64, 0:16],
            in1=At[0:64, 16:32],
            op=ALU.subtract,
        )
        # v/side diffs vs host targets on rows 64 and 96
        nc.vector.tensor_tensor(
            out=Dt[64:128, :],
            in0=At[64:128, :],
            in1=HDt[64:128, 0:32],
            op=ALU.subtract,
        )

        RESt = pool.tile([128, 32], f32)
        # CE terms: softplus(d) = ln(1 + exp(d)); the +1 rides the Ln bias
        E = pool.tile([128, 16], f32)
        nc.scalar.activation(E[0:64, :], Dt[0:64, 0:16], AF.Exp)
        nc.scalar.activation(RESt[0:64, 0:16], E[0:64, :], AF.Ln, bias=1.0)
        # smooth-L1: 0.5*min(|d|,1)^2 + |d| - min(|d|,1)
        Aab = pool.tile([128, 32], f32)
        nc.scalar.activation(Aab[64:128, :], Dt[64:128, :], AF.Abs)
        Mn = pool.tile([128, 32], f32)
        nc.vector.tensor_scalar_min(Mn[64:128, :], Aab[64:128, :], 1.0)
        T = pool.tile([128, 32], f32)
        nc.vector.tensor_tensor(
            out=T[64:128, :], in0=Aab[64:128, :], in1=Mn[64:128, :],
            op=ALU.subtract,
        )
        Q = pool.tile([128, 32], f32)
        # Square(scale*x) with scale=sqrt(0.5) gives 0.5*x^2
        nc.scalar.activation(
            Q[64:128, :], Mn[64:128, :], AF.Square, scale=0.7071067811865476
        )
        nc.vector.tensor_tensor(
            out=RESt[64:128, :], in0=Q[64:128, :], in1=T[64:128, :],
            op=ALU.add,
        )
        # count slots ride in the dead half of the side row
        nc.vector.memset(RESt[96:97, 16:32], 1.0)

        TWt = pool.tile([128, 32], f32)
        nc.vector.tensor_tensor(
            out=TWt[0:64, 0:16],
            in0=RESt[0:64, 0:16],
            in1=HDt[0:64, 32:48],
            op=ALU.mult,
        )
        nc.vector.tensor_tensor(
            out=TWt[64:128, :],
            in0=RESt[64:128, :],
            in1=HDt[64:128, 32:64],
            op=ALU.mult,
        )

        St = pool.tile([128, 2], f32)
        nc.vector.tensor_reduce(
            out=St[0:64, 0:1],
            in_=TWt[0:64, 0:16].rearrange("p (s x) -> p s x", x=16),
            axis=mybir.AxisListType.X,
            op=ALU.add,
        )
        nc.vector.tensor_reduce(
            out=St[64:128, :],
            in_=TWt[64:128, :].rearrange("p (s x) -> p s x", x=16),
            axis=mybir.AxisListType.X,
            op=ALU.add,
        )
        nc.vector.memset(St[0:64, 1:2], 0.0)
        nc.sync.dma_start(out.ap(), St[0:97:32, :])

    nc.compile()
    return nc


def _get_nc():
    if "nc" not in _CACHE:
        _CACHE["nc"] = _build_nc()
    return _CACHE["nc"]


def _pack_core_inputs(
    scores, vcoords, sides, pos_y, pos_x, pos_z, neg_y, neg_x, neg_z,
    v_targets, side_mask, side_targets,
):
    """Build the 8 per-core input maps (replicated shards + anchor meta)."""
    scores = np.asarray(scores, dtype=np.float32).reshape(2 * K, H, W)
    vcoords = np.asarray(vcoords, dtype=np.float32).reshape(2 * K, H, W)
    sides = np.asarray(sides, dtype=np.float32).reshape(K, H, W)
    pos_y = np.asarray(pos_y).astype(np.int64)
    pos_x = np.asarray(pos_x).astype(np.int64)
    pos_z = np.asarray(pos_z).astype(np.int64)
    neg_y = np.asarray(neg_y).astype(np.int64)
    neg_x = np.asarray(neg_x).astype(np.int64)
    neg_z = np.asarray(neg_z).astype(np.int64)
    v_targets = np.asarray(v_targets, dtype=np.float32)
    side_mask_f = np.asarray(side_mask).astype(np.float32)
    side_targets = np.asarray(side_targets, dtype=np.float32)

    pshard = pos_y // HSH
    nshard = neg_y // HSH
    pbase = (pos_y % HSH) * W + pos_x
    nbase = (neg_y % HSH) * W + neg_x
    p_s0 = SCB + (2 * pos_z) * PLANE + pbase
    p_s1 = SCB + (2 * pos_z + 1) * PLANE + pbase
    p_v0 = VCB + (2 * pos_z) * PLANE + pbase
    p_v1 = VCB + (2 * pos_z + 1) * PLANE + pbase
    p_sd = SDB + pos_z * PLANE + pbase
    n_s0 = SCB + (2 * neg_z) * PLANE + nbase
    n_s1 = SCB + (2 * neg_z + 1) * PLANE + nbase

    shard_data = []
    for s in range(N_SHARDS):
        r = slice(s * HSH, (s + 1) * HSH)
        shard_data.append(
            np.concatenate(
                [
                    scores[:, r, :].reshape(-1),
                    vcoords[:, r, :].reshape(-1),
                    sides[:, r, :].reshape(-1),
                ]
            ).reshape(DN, 1)
        )

    in_maps = []
    for c in range(N_CORES):
        s, lane = divmod(c, LANES)
        sel_p = np.nonzero(pshard == s)[0][lane::LANES]
        sel_n = np.nonzero(nshard == s)[0][lane::LANES]
        npc, nnc = len(sel_p), len(sel_n)
        assert npc <= CAP and nnc <= CAP

        idx = np.zeros((128, 1), dtype=np.int32)
        idx[0:npc, 0] = p_s0[sel_p]
        idx[16 : 16 + npc, 0] = p_s1[sel_p]
        idx[32 : 32 + nnc, 0] = n_s1[sel_n]
        idx[48 : 48 + nnc, 0] = n_s0[sel_n]
        idx[64 : 64 + npc, 0] = p_v0[sel_p]
        idx[80 : 80 + npc, 0] = p_v1[sel_p]
        idx[96 : 96 + npc, 0] = p_sd[sel_p]

        # rows = partitions; data rows {0,32,64,96}; cols 0:32 targets,
        # 32:64 weights; all other cells zero
        hd = np.zeros((128, 64), dtype=np.float32)
        hd[0, 32 : 32 + npc] = 1.0                       # cls pos weight
        hd[32, 32 : 32 + nnc] = 1.0                      # cls neg weight
        hd[64, 0:npc] = v_targets[sel_p, 0]              # vt0
        hd[64, 16 : 16 + npc] = v_targets[sel_p, 1]      # vt1
        hd[64, 32 : 32 + npc] = 1.0                      # v0 weight
        hd[64, 48 : 48 + npc] = 1.0                      # v1 weight
        hd[96, 0:npc] = side_targets[sel_p]              # st
        hd[96, 32 : 32 + npc] = side_mask_f[sel_p]       # o term weight
        hd[96, 48 : 48 + npc] = side_mask_f[sel_p]       # count weight

        in_maps.append({"data": shard_data[s], "idx": idx, "hostdat": hd})
    return in_maps


def _finalize(partials_list):
    """Combine per-core partial sums into the 4 reference outputs.

    partials rows: 0 (cls_pos, 0) | 1 (cls_neg, 0) | 2 (v0_sum, v1_sum) |
                   3 (o_sum, count)
    """
    S = np.zeros((4, 2), dtype=np.float64)
    for p in partials_list:
        S += np.asarray(p).reshape(4, 2).astype(np.float64)
    cls = (S[0, 0] + S[1, 0]) / (N_POS + N_NEG)
    reg_v = (S[2, 0] + S[2, 1]) / (N_POS * 2)
    cnt = S[3, 1]
    reg_o = (S[3, 0] / max(cnt, 1.0)) if cnt > 0 else 0.0
    loss = cls + 1.0 * reg_v + 2.0 * reg_o
    return (
        np.float32(loss),
        np.float32(cls),
        np.float32(reg_v),
        np.float32(reg_o),
    )


def _install_ntff_hook():
    """The agent image's antenv lacks axon_hooks; synthesize it and wire the
    ctypes NTFF profiling hook from trn_boot so trace=True works."""
    import sys
    import types

    if "antenv.axon_hooks" in sys.modules:
        return True
    try:
        from trn_agent_boot.trn_boot import _ntff_profile_via_ctypes

        hook = _ntff_profile_via_ctypes("/opt/axon/libaxon_pjrt.so")
        if hook is None:
            return False
        mod = types.ModuleType("antenv.axon_hooks")
        mod._hook = hook
        mod.get_axon_ntff_profile_hook = lambda: mod._hook

        def _set(h):
            mod._hook = h

        mod.set_axon_ntff_profile_hook = _set
        sys.modules["antenv.axon_hooks"] = mod
        return True
    except Exception:
        return False


def kernel(**inputs):
    global last_exec_time_ns
    from concourse.bass_utils import run_bass_kernel_spmd

    nc = _get_nc()
    in_maps = _pack_core_inputs(**inputs)

    trace = os.environ.get("KERNEL_PROFILE", "") == "1" and _install_ntff_hook()
    res = run_bass_kernel_spmd(
        nc, in_maps, list(range(N_CORES)), trace=trace
    )
    last_exec_time_ns = res.exec_time_ns

    return _finalize([r["partials"] for r in res.results])


# revision 23
# speedup vs baseline: 1.0791x; 1.0791x over previous
"""CTPN loss kernel for Trainium2 (8 NeuronCores, Bass/Tile).

Strategy
--------
The loss only touches 64 pos + 64 neg anchor locations of the dense
[1,2K,H,W] maps, so the kernel is a sparse gather + tiny reduction.

- Dense tensors are sharded by image rows into 2 shards of 256 rows;
  each shard is held by 4 cores (data-parallel replicas). scores,
  vcoords and sides shards are concatenated into ONE flat per-core
  "data" tensor so every gathered element addresses a single tensor.
- Anchors are routed to the 4 cores holding their rows (round-robin),
  so each core owns at most 16 pos + 16 neg anchors. All 112 values a
  core needs (5 per pos anchor slot, 2 per neg slot) are fetched from
  HBM with a single [128,1] indirect DMA (partition-major, the only
  layout the hardware DGE gather supports), then pivoted on-chip with
  one DVE 32x32 block transpose: slot block i (32 slots) lands on
  partition 32*i as a free-dim row of 32. Minuends and subtrahends
  share a block, so every downstream op is a plain free-dim slice at a
  legal partition start (0/32/64/96) with uniform offsets.
- Loss terms: CE via softplus(d) = ln(1+exp(d)), smooth-L1 via
  0.5*min(|d|,1)^2 + |d| - min(|d|,1). Terms are weighted (anchor
  validity / side mask) and block-reduced with two 16-wide X-reduces
  into 8 partial sums per core. The 8 partial vectors are summed on
  host (the all-reduce of the scalar losses) and normalized into the
  4 outputs.

Gather slot layout (16 slots per half-block; block i -> partition 32i):
  [0:16)    s0p   [16:32)  s1p     (pos CE pair        -> row 0)
  [32:48)   s1n   [48:64)  s0n     (neg CE pair        -> row 32)
  [64:80)   v0    [80:96)  v1      (v-coord minuends   -> row 64)
  [96:112)  sd    [112:128) pad    (side minuends/cnt  -> row 96)

The compiled program is input-independent (offsets travel as data), so
it is built and compiled once and reused across calls.
"""

import os

import numpy as np

H, W, K = 512, 1024, 10
N_POS = 64
N_NEG = 64
N_CORES = 8
N_SHARDS = 2
LANES = N_CORES // N_SHARDS  # replicas per shard
HSH = H // N_SHARDS          # 256 rows per shard
PLANE = HSH * W              # elements per channel per shard
SCB = 0                      # scores base within concat data
VCB = 2 * K * PLANE          # vcoords base
SDB = 4 * K * PLANE          # sides base
DN = 5 * K * PLANE           # total concat length per core
CAP = 16                     # per-core anchor capacity (exact worst case)

_CACHE = {}
last_exec_time_ns = None


def _build_nc():
    import concourse.bass as bass
    import concourse.tile as tile
    from concourse import bacc, mybir
    from contextlib import ExitStack

    f32 = mybir.dt.float32
    i32 = mybir.dt.int32
    AF = mybir.ActivationFunctionType
    ALU = mybir.AluOpType

    nc = bacc.Bacc(
        "TRN2", target_bir_lowering=False, debug=False, num_devices=N_CORES
    )

    data = nc.dram_tensor("data", [DN, 1], f32, kind="ExternalInput")
    idx = nc.dram_tensor("idx", [128, 1], i32, kind="ExternalInput")
    # hostdat rows = partitions; data rows {0,32,64,96}, cols [0:32) targets,
    # [32:64) weights (aligned with the transposed data rows), zero elsewhere
    hd = nc.dram_tensor("hostdat", [128, 64], f32, kind="ExternalInput")
    out = nc.dram_tensor("partials", [4, 2], f32, kind="ExternalOutput")

    with ExitStack() as ctx:
        tc = ctx.enter_context(tile.TileContext(nc))
        pool = ctx.enter_context(tc.tile_pool(name="p", bufs=1))

        idx_t = pool.tile([128, 1], i32)
        # idx rides the sync (HWDGE) queue first: it issues ~0.7us earlier
        # than a gpsimd-issued load and everything downstream gates on it
        nc.sync.dma_start(idx_t[:], idx.ap())
        HDt = pool.tile([128, 64], f32)
        nc.sync.dma_start(HDt[:], hd.ap())

        A32 = pool.tile([128, 32], f32)
        nc.vector.memset(A32[:, 1:32], 0.0)
        nc.gpsimd.indirect_dma_start(
            out=A32[:, 0:1],
            out_offset=None,
            in_=data.ap(),
            in_offset=bass.IndirectOffsetOnAxis(ap=idx_t[:], axis=0),
        )
        # pivot: 32x32 block transpose puts slot block i on partition 32i
        # (all other rows of At are exact zeros, so batched ops over the
        # full [0:64) / [64:128) partition ranges stay finite and the
        # zero weights kill their contributions)
        At = pool.tile([128, 32], f32)
        nc.vector.transpose(At[:], A32[:])

        Dt = pool.tile([128, 32], f32)
        # CE diffs: minuend - subtrahend halves of rows 0 and 32
        nc.vector.tensor_tensor(
            out=Dt[0:64, 0:16],
            in0=At[0:64, 0:16],
            in1=At[0:64, 16:32],
            op=ALU.subtract,
        )
        # v/side diffs vs host targets on rows 64 and 96
        nc.vector.tensor_tensor(
            out=Dt[64:128, :],
            in0=At[64:128, :],
            in1=HDt[64:128, 0:32],
            op=ALU.subtract,
        )

        RESt = pool.tile([128, 32], f32)
        # CE terms: softplus(d) = ln(1 + exp(d)); the +1 rides the Ln bias
        E = pool.tile([128, 16], f32)
        nc.scalar.activation(E[0:64, :], Dt[0:64, 0:16], AF.Exp)
        nc.scalar.activation(RESt[0:64, 0:16], E[0:64, :], AF.Ln, bias=1.0)
        # smooth-L1: 0.5*min(|d|,1)^2 + |d| - min(|d|,1)
        Aab = pool.tile([128, 32], f32)
        nc.scalar.activation(Aab[64:128, :], Dt[64:128, :], AF.Abs)
        Mn = pool.tile([128, 32], f32)
        nc.vector.tensor_scalar_min(Mn[64:128, :], Aab[64:128, :], 1.0)
        T = pool.tile([128, 32], f32)
        nc.vector.tensor_tensor(
            out=T[64:128, :], in0=Aab[64:128, :], in1=Mn[64:128, :],
            op=ALU.subtract,
        )
        Q = pool.tile([128, 32], f32)
        # Square(scale*x) with scale=sqrt(0.5) gives 0.5*x^2
        nc.scalar.activation(
            Q[64:128, :], Mn[64:128, :], AF.Square, scale=0.7071067811865476
        )
        nc.vector.tensor_tensor(
            out=RESt[64:128, :], in0=Q[64:128, :], in1=T[64:128, :],
            op=ALU.add,
        )
        # count slots ride in the dead half of the side row
        nc.vector.memset(RESt[96:97, 16:32], 1.0)

        TWt = pool.tile([128, 32], f32)
        nc.vector.tensor_tensor(
            out=TWt[0:64, 0:16],
            in0=RESt[0:64, 0:16],
            in1=HDt[0:64, 32:48],
            op=ALU.mult,
        )
        nc.vector.tensor_tensor(
            out=TWt[64:128, :],
            in0=RESt[64:128, :],
            in1=HDt[64:128, 32:64],
            op=ALU.mult,
        )

        St = pool.tile([128, 2], f32)
        nc.vector.tensor_reduce(
            out=St[0:64, 0:1],
            in_=TWt[0:64, 0:16].rearrange("p (s x) -> p s x", x=16),
            axis=mybir.AxisListType.X,
            op=ALU.add,
        )
        nc.vector.tensor_reduce(
            out=St[64:128, :],
            in_=TWt[64:128, :].rearrange("p (s x) -> p s x", x=16),
            axis=mybir.AxisListType.X,
            op=ALU.add,
        )
        nc.vector.memset(St[0:64, 1:2], 0.0)
        nc.sync.dma_start(out.ap(), St[0:97:32, :])

    nc.compile()
    return nc


def _get_nc():
    if "nc" not in _CACHE:
        _CACHE["nc"] = _build_nc()
    return _CACHE["nc"]


def _pack_core_inputs(
    scores, vcoords, sides, pos_y, pos_x, pos_z, neg_y, neg_x, neg_z,
    v_targets, side_mask, side_targets,
):
    """Build the 8 per-core input maps (replicated shards + anchor meta)."""
    scores = np.asarray(scores, dtype=np.float32).reshape(2 * K, H, W)
    vcoords = np.asarray(vcoords, dtype=np.float32).reshape(2 * K, H, W)
    sides = np.asarray(sides, dtype=np.float32).reshape(K, H, W)
    pos_y = np.asarray(pos_y).astype(np.int64)
    pos_x = np.asarray(pos_x).astype(np.int64)
    pos_z = np.asarray(pos_z).astype(np.int64)
    neg_y = np.asarray(neg_y).astype(np.int64)
    neg_x = np.asarray(neg_x).astype(np.int64)
    neg_z = np.asarray(neg_z).astype(np.int64)
    v_targets = np.asarray(v_targets, dtype=np.float32)
    side_mask_f = np.asarray(side_mask).astype(np.float32)
    side_targets = np.asarray(side_targets, dtype=np.float32)

    pshard = pos_y // HSH
    nshard = neg_y // HSH
    pbase = (pos_y % HSH) * W + pos_x
    nbase = (neg_y % HSH) * W + neg_x
    p_s0 = SCB + (2 * pos_z) * PLANE + pbase
    p_s1 = SCB + (2 * pos_z + 1) * PLANE + pbase
    p_v0 = VCB + (2 * pos_z) * PLANE + pbase
    p_v1 = VCB + (2 * pos_z + 1) * PLANE + pbase
    p_sd = SDB + pos_z * PLANE + pbase
    n_s0 = SCB + (2 * neg_z) * PLANE + nbase
    n_s1 = SCB + (2 * neg_z + 1) * PLANE + nbase

    shard_data = []
    for s in range(N_SHARDS):
        r = slice(s * HSH, (s + 1) * HSH)
        shard_data.append(
            np.concatenate(
                [
                    scores[:, r, :].reshape(-1),
                    vcoords[:, r, :].reshape(-1),
                    sides[:, r, :].reshape(-1),
                ]
            ).reshape(DN, 1)
        )

    in_maps = []
    for c in range(N_CORES):
        s, lane = divmod(c, LANES)
        sel_p = np.nonzero(pshard == s)[0][lane::LANES]
        sel_n = np.nonzero(nshard == s)[0][lane::LANES]
        npc, nnc = len(sel_p), len(sel_n)
        assert npc <= CAP and nnc <= CAP

        idx = np.zeros((128, 1), dtype=np.int32)
        idx[0:npc, 0] = p_s0[sel_p]
        idx[16 : 16 + npc, 0] = p_s1[sel_p]
        idx[32 : 32 + nnc, 0] = n_s1[sel_n]
        idx[48 : 48 + nnc, 0] = n_s0[sel_n]
        idx[64 : 64 + npc, 0] = p_v0[sel_p]
        idx[80 : 80 + npc, 0] = p_v1[sel_p]
        idx[96 : 96 + npc, 0] = p_sd[sel_p]

        # rows = partitions; data rows {0,32,64,96}; cols 0:32 targets,
        # 32:64 weights; all other cells zero
        hd = np.zeros((128, 64), dtype=np.float32)
        hd[0, 32 : 32 + npc] = 1.0                       # cls pos weight
        hd[32, 32 : 32 + nnc] = 1.0                      # cls neg weight
        hd[64, 0:npc] = v_targets[sel_p, 0]              # vt0
        hd[64, 16 : 16 + npc] = v_targets[sel_p, 1]      # vt1
        hd[64, 32 : 32 + npc] = 1.0                      # v0 weight
        hd[64, 48 : 48 + npc] = 1.0                      # v1 weight
        hd[96, 0:npc] = side_targets[sel_p]              # st
        hd[96, 32 : 32 + npc] = side_mask_f[sel_p]       # o term weight
        hd[96, 48 : 48 + npc] = side_mask_f[sel_p]       # count weight

        in_maps.append({"data": shard_data[s], "idx": idx, "hostdat": hd})
    return in_maps


def _finalize(partials_list):
    """Combine per-core partial sums into the 4 reference outputs.

    partials rows: 0 (cls_pos, 0) | 1 (cls_neg, 0) | 2 (v0_sum, v1_sum) |
                   3 (o_sum, count)
    """
    S = np.zeros((4, 2), dtype=np.float64)
    for p in partials_list:
        S += np.asarray(p).reshape(4, 2).astype(np.float64)
    cls = (S[0, 0] + S[1, 0]) / (N_POS + N_NEG)
    reg_v = (S[2, 0] + S[2, 1]) / (N_POS * 2)
    cnt = S[3, 1]
    reg_o = (S[3, 0] / max(cnt, 1.0)) if cnt > 0 else 0.0
    loss = cls + 1.0 * reg_v + 2.0 * reg_o
    return (
        np.float32(loss),
        np.float32(cls),
        np.float32(reg_v),
        np.float32(reg_o),
    )


def _install_ntff_hook():
    """The agent image's antenv lacks axon_hooks; synthesize it and wire the
    ctypes NTFF profiling hook from trn_boot so trace=True works."""
    import sys
    import types

    if "antenv.axon_hooks" in sys.modules:
        return True
    try:
        from trn_agent_boot.trn_boot import _ntff_profile_via_ctypes

        hook = _ntff_profile_via_ctypes("/opt/axon/libaxon_pjrt.so")
        if hook is None:
            return False
        mod = types.ModuleType("antenv.axon_hooks")
        mod._hook = hook
        mod.get_axon_ntff_profile_hook = lambda: mod._hook

        def _set(h):
            mod._hook = h

        mod.set_axon_ntff_profile_hook = _set
        sys.modules["antenv.axon_hooks"] = mod
        return True
    except Exception:
        return False


def kernel(**inputs):
    global last_exec_time_ns
    from concourse.bass_utils import run_bass_kernel_spmd

    nc = _get_nc()
    in_maps = _pack_core_inputs(**inputs)

    trace = os.environ.get("KERNEL_PROFILE", "") == "1" and _install_ntff_hook()
    res = run_bass_kernel_spmd(
        nc, in_maps, list(range(N_CORES)), trace=trace
    )
    last_exec_time_ns = res.exec_time_ns

    return _finalize([r["partials"] for r in res.results])
